# revision 1
# baseline (speedup 1.0000x reference)
"""Trainium2 Bass kernel for CachedRoPEAttention.

Sharding: 8 cores = batch(2) x head-groups(4). Each core computes 4 heads of
one batch element end-to-end (q/k/v proj in [e,t] layout, RoPE, causal
flash-style attention with ones-row softmax denominators, out_proj partial),
host sums the 4 tensor-parallel partials per batch.

All matmuls run in float32r (fp32 bits, HW rounds stream to ~12-bit mantissa,
1 cycle/row at N>=256).
"""
import sys
sys.path.insert(0, "/opt/trn_rl_repo")

import numpy as np

import concourse.bass as bass
import concourse.bacc as bacc
import concourse.mybir as mybir
import concourse.tile as tile
from concourse.bass_utils import run_bass_kernel_spmd

F32 = mybir.dt.float32
F32R = mybir.dt.float32r

D, H, DH, T, B = 1024, 16, 64, 2048, 2
HG, HPC, EC = 4, 4, 256      # head groups, heads/core, e-width/core
KT = D // 128                # 8 contraction tiles over d_model
PT = EC // 128               # 2 e-partition-tiles per core
NB = T // 512                # 4 t-blocks
NTT = T // 128               # 16 t-tiles

_NC_CACHE = {}


def _build_nc():
    nc = bacc.Bacc(None, target_bir_lowering=False)

    xT_d = nc.dram_tensor("xT", [D, T], F32R, kind="ExternalInput")
    wqT_d = nc.dram_tensor("wqT", [D, EC], F32R, kind="ExternalInput")
    wkT_d = nc.dram_tensor("wkT", [D, EC], F32R, kind="ExternalInput")
    wvT_d = nc.dram_tensor("wvT", [D, EC], F32R, kind="ExternalInput")
    woT_d = nc.dram_tensor("woT", [EC, D], F32R, kind="ExternalInput")
    cos2_d = nc.dram_tensor("cos2", [128, T], F32, kind="ExternalInput")
    sin2p_d = nc.dram_tensor("sin2p", [128, T], F32, kind="ExternalInput")
    tri_d = nc.dram_tensor("tri", [128, 128], F32R, kind="ExternalInput")
    ztri_d = nc.dram_tensor("ztri", [128, 256], F32R, kind="ExternalInput")
    ones_d = nc.dram_tensor("ones", [128, 1], F32R, kind="ExternalInput")
    outT_d = nc.dram_tensor("outT", [D, T], F32, kind="ExternalOutput")

    with tile.TileContext(nc) as tc:
        with tc.tile_pool(name="perm", bufs=1) as perm, \
             tc.tile_pool(name="psum", bufs=1, space="PSUM") as psp, \
             tc.tile_pool(name="dram", bufs=2, space="DRAM") as drp:
            # ---- persistent tiles
            qT = perm.tile([128, PT, T], F32R)
            kT = perm.tile([128, PT, T], F32R)
            v_sb = perm.tile([128, NTT, HPC, 65], F32R)
            wo_sb = perm.tile([128, 2, D], F32R)
            tri_sb = perm.tile([128, 128], F32R)
            ztri_sb = perm.tile([128, 256], F32R)
            ones_sb = perm.tile([33, 64], F32R)
            # ================= phase 1: projections + RoPE =================
            with tc.tile_pool(name="ph1", bufs=1) as ph1, \
                 tc.tile_pool(name="rw", bufs=3) as rw:
                x_sb = ph1.tile([128, KT, T], F32R)
                wq_sb = ph1.tile([128, KT, EC], F32R)
                wk_sb = ph1.tile([128, KT, EC], F32R)
                wv_sb = ph1.tile([128, KT, EC], F32R)
                cos_sb = ph1.tile([128, T], F32)
                sin_sb = ph1.tile([128, T], F32)
                # load order: x/wv first (v proj runs first), alternating the
                # two HWDGE queues (SP + ACT); consts/wo deferred to the tail
                for k in range(KT):
                    r = slice(128 * k, 128 * k + 128)
                    nc.sync.dma_start(out=x_sb[:, k, :], in_=xT_d.ap()[r, :])
                    nc.scalar.dma_start(out=wv_sb[:, k, :], in_=wvT_d.ap()[r, :])
                for k in range(KT):
                    r = slice(128 * k, 128 * k + 128)
                    nc.sync.dma_start(out=wq_sb[:, k, :], in_=wqT_d.ap()[r, :])
                    nc.scalar.dma_start(out=wk_sb[:, k, :], in_=wkT_d.ap()[r, :])
                nc.sync.dma_start(out=cos_sb, in_=cos2_d.ap())
                nc.scalar.dma_start(out=sin_sb, in_=sin2p_d.ap())
                nc.sync.dma_start(out=tri_sb, in_=tri_d.ap())
                nc.scalar.dma_start(out=ztri_sb, in_=ztri_d.ap())
                for ct in range(2):
                    nc.scalar.dma_start(out=wo_sb[:, ct, :],
                                        in_=woT_d.ap()[128 * ct:128 * ct + 128, :])
                ones_bcast = bass.AP(tensor=ones_d, offset=0,
                                     ap=[[1, 128], [0, NTT * HPC]])
                nc.sync.dma_start(
                    out=v_sb[:, :, :, 64:65].rearrange("p a b c -> p (a b c)"),
                    in_=ones_bcast)
                ones_row = bass.AP(tensor=ones_d, offset=0, ap=[[64, 1], [1, 64]])
                nc.sync.dma_start(out=ones_sb[0:1, :], in_=ones_row)
                nc.sync.dma_start(out=ones_sb[32:33, :], in_=ones_row)

                # v projection first (attention needs v earliest)
                for tt in range(NTT):
                    ps = psp.tile([128, 256], F32, tag="mm", bufs=2, name=f"psv{tt}")
                    for k in range(KT):
                        nc.tensor.matmul(
                            ps, x_sb[:, k, 128 * tt:128 * tt + 128],
                            wv_sb[:, k, :],
                            start=(k == 0), stop=(k == KT - 1))
                    nc.vector.tensor_copy(
                        out=v_sb[:, tt, :, 0:64],
                        in_=ps.rearrange("p (h d) -> p h d", h=HPC))

                # q/k projections + fused RoPE, chunk = [128, 512];
                # pair-0 chunks first so attention can start early
                for pt in range(PT):
                    for nb in range(NB):
                        cols = slice(512 * nb, 512 * nb + 512)
                        for w_sb, dst, wnm in ((wq_sb, qT, "q"), (wk_sb, kT, "k")):
                            ps = psp.tile([128, 512], F32, tag="mm", bufs=2,
                                          name=f"ps{wnm}{pt}{nb}")
                            for k in range(KT):
                                nc.tensor.matmul(
                                    ps, w_sb[:, k, 128 * pt:128 * pt + 128],
                                    x_sb[:, k, cols],
                                    start=(k == 0), stop=(k == KT - 1))
                            qc = rw.tile([128, 512], F32R, tag="qc")
                            nc.vector.tensor_mul(out=qc, in0=ps, in1=cos_sb[:, cols])
                            # sin term: multiply by pre-shuffled sin const,
                            # THEN partition-shuffle the SBUF product via DMA
                            qp = rw.tile([128, 512], F32R, tag="qp")
                            nc.vector.tensor_mul(out=qp, in0=ps, in1=sin_sb[:, cols])
                            shuf = rw.tile([128, 512], F32R, tag="shuf")
                            for b2 in range(2):
                                base = 64 * b2
                                nc.sync.dma_start(out=shuf[base:base + 32, :],
                                                  in_=qp[base + 32:base + 64, :])
                                nc.sync.dma_start(out=shuf[base + 32:base + 64, :],
                                                  in_=qp[base:base + 32, :])
                            nc.vector.tensor_add(out=dst[:, pt, cols], in0=qc, in1=shuf)

            # ================= phase 2: attention + out_proj =================
            with tc.tile_pool(name="att", bufs=1) as att, \
                 tc.tile_pool(name="ew", bufs=3) as ew:
                OT_all = att.tile([128, PT, T], F32R)
                for blk in range(NB):
                    cols = slice(512 * blk, 512 * blk + 512)
                    for pair in range(PT):
                        prow = slice(0, 128)
                        ot = [psp.tile([65, 512], F32, tag="ot", bufs=2,
                                       name=f"ot{blk}{pair}{hd}")
                              for hd in range(2)]
                        ntk = 2 * (blk + 1)
                        last = (ntk - 1, 1)
                        for tkp in range(ntk):
                            t0 = 2 * tkp
                            lo = [max(0, 128 * (t0 + h - 4 * blk)) for h in (0, 1)]
                            lop = lo[0]
                            st = [psp.tile([128, 2, 512], F32, tag="st2", bufs=2,
                                           name=f"st{blk}{pair}{tkp}{hd}")
                                  for hd in range(2)]
                            for hd in range(2):
                                hrow = slice(64 * hd, 64 * hd + 64)
                                for h in (0, 1):
                                    tt = t0 + h
                                    nc.tensor.matmul(
                                        st[hd][:, h, lop:512],
                                        kT[hrow, pair, 128 * tt:128 * tt + 128],
                                        qT[hrow, pair, 512 * blk + lop:512 * blk + 512],
                                        start=True, stop=True)
                            ex = [ew.tile([128, 2, 512], F32R, tag="ex",
                                          name=f"ex{blk}{pair}{tkp}{hd2}")
                                  for hd2 in range(2)]
                            for hd in range(2):
                                nc.scalar.activation(
                                    out=ex[hd][:, :, lop:512],
                                    in_=st[hd][:, :, lop:512],
                                    func=mybir.ActivationFunctionType.Exp,
                                    scale=0.125)
                                # causal masks on diagonal tiles
                                for h in (0, 1):
                                    j = t0 + h - 4 * blk
                                    if j < 0:
                                        continue
                                    lo_h = lo[h]
                                    if lo_h == lop:
                                        nc.vector.tensor_mul(
                                            out=ex[hd][:, h, lo_h:lo_h + 128],
                                            in0=ex[hd][:, h, lo_h:lo_h + 128],
                                            in1=tri_sb)
                                    else:
                                        w = lo_h + 128 - lop
                                        nc.vector.tensor_mul(
                                            out=ex[hd][:, h, lop:lo_h + 128],
                                            in0=ex[hd][:, h, lop:lo_h + 128],
                                            in1=ztri_sb[:, 0:w])
                                for h in (0, 1):
                                    tt = t0 + h
                                    lo_h = lo[h]
                                    nc.tensor.matmul(
                                        ot[hd][:, lo_h:512],
                                        v_sb[:, tt, 2 * pair + hd, :],
                                        ex[hd][:, h, lo_h:512],
                                        start=(tkp == 0 and h == 0),
                                        stop=(tkp, h) == last)
                        # copy raw OT out of PSUM promptly (frees the banks),
                        # then denominators -> DRAM-bounce broadcast -> normalize
                        ots = [ew.tile([65, 512], F32, tag="ots", bufs=2,
                                       name=f"ots{blk}{pair}{hd}")
                               for hd in range(2)]
                        for hd in range(2):
                            nc.vector.tensor_copy(out=ots[hd], in_=ot[hd])
                        # reciprocal -> PE ones-outer-product broadcast to 64
                        # partitions in PSUM -> normalize muls read it directly
                        rc2 = ew.tile([33, 512], F32R, tag="rc2",
                                      name=f"rc{blk}{pair}")
                        with nc.allow_low_precision(reason="f32r recip feeds bcast matmul"):
                            nc.vector.reciprocal(out=rc2[0:1, :], in_=ots[0][64:65, :])
                            nc.vector.reciprocal(out=rc2[32:33, :], in_=ots[1][64:65, :])
                        rcp = [psp.tile([64, 512], F32, tag="ot", bufs=2,
                                        name=f"rcp{blk}{pair}{hd}")
                               for hd in range(2)]
                        nc.tensor.matmul(rcp[0], ones_sb[0:1, :], rc2[0:1, :],
                                         start=True, stop=True)
                        nc.tensor.matmul(rcp[1], ones_sb[32:33, :], rc2[32:33, :],
                                         start=True, stop=True)
                        nc.vector.tensor_mul(out=OT_all[0:64, pair, cols],
                                             in0=ots[0][0:64, :], in1=rcp[0])
                        nc.vector.tensor_mul(out=OT_all[64:128, pair, cols],
                                             in0=ots[1][0:64, :], in1=rcp[1])
                    # out_proj for this t-block
                    for m in range(KT):
                        fp = psp.tile([128, 512], F32, tag="mm", bufs=2,
                                      name=f"fp{blk}{m}")
                        for ct in range(2):
                            nc.tensor.matmul(
                                fp, wo_sb[:, ct, 128 * m:128 * m + 128],
                                OT_all[:, ct, cols],
                                start=(ct == 0), stop=(ct == 1))
                        fs = ew.tile([128, 512], F32, tag="fs",
                                     name=f"fs{blk}{m}")
                        nc.vector.tensor_copy(out=fs, in_=fp)
                        nc.sync.dma_start(
                            out=outT_d.ap()[128 * m:128 * m + 128, cols], in_=fs)

    nc.compile()
    return nc


def _consts():
    i = np.arange(32)
    theta = 1.0 / (10000.0 ** (2.0 * i / 64))
    ang = np.outer(np.arange(T, dtype=np.float64), theta)
    p = np.arange(128)
    cos2 = np.cos(ang[:, p % 32]).T.astype(np.float32)
    sgn = np.where((p % 64) < 32, -1.0, 1.0)
    sin2s = (np.sin(ang[:, p % 32]) * sgn).T.astype(np.float32)
    cos2 = np.ascontiguousarray(cos2)
    # pre-shuffled sin so the kernel can multiply BEFORE the partition shuffle:
    # shuf(q * sin2p)[p] = q[p^32] * sin2s[p]
    sin2p = np.ascontiguousarray(sin2s[p ^ 32])
    r, c = np.meshgrid(np.arange(128), np.arange(128), indexing="ij")
    tri = (r <= c).astype(np.float32)
    ztri = np.ascontiguousarray(
        np.concatenate([np.zeros((128, 128), np.float32), tri], axis=1))
    ones = np.ones((128, 1), np.float32)
    return cos2, sin2p, tri, ztri, ones


def kernel(x, Wq, Wk, Wv, Wo, _trace=False):
    x = np.asarray(x, dtype=np.float32)
    Wq = np.asarray(Wq, dtype=np.float32)
    Wk = np.asarray(Wk, dtype=np.float32)
    Wv = np.asarray(Wv, dtype=np.float32)
    Wo = np.asarray(Wo, dtype=np.float32)

    if "nc" not in _NC_CACHE:
        _NC_CACHE["nc"] = _build_nc()
    nc = _NC_CACHE["nc"]

    cos2, sin2p, tri, ztri, ones = _consts()
    xTs = [np.ascontiguousarray(x[b].T) for b in range(B)]
    WqT, WkT, WvT, WoT = Wq.T, Wk.T, Wv.T, Wo.T

    in_maps = []
    for c in range(8):
        b, g = c // HG, c % HG
        cs = slice(EC * g, EC * g + EC)
        in_maps.append({
            "xT": xTs[b],
            "wqT": np.ascontiguousarray(WqT[:, cs]),
            "wkT": np.ascontiguousarray(WkT[:, cs]),
            "wvT": np.ascontiguousarray(WvT[:, cs]),
            "woT": np.ascontiguousarray(WoT[cs, :]),
            "cos2": cos2, "sin2p": sin2p,
            "tri": tri, "ztri": ztri, "ones": ones,
        })

    kw = {}
    if _trace:
        kw = dict(trace=True, trace_cores=list(range(8)))
    res = run_bass_kernel_spmd(nc, in_maps, core_ids=list(range(8)), **kw)

    out = np.zeros((B, T, D), np.float32)
    for c in range(8):
        out[c // HG] += res.results[c]["outT"].T
    if _trace:
        return out, res
    return out

